# revision 19
# baseline (speedup 1.0000x reference)
"""LeViT-style attention (B=32, N=784, DIM=512, H=8, KD=32, VD=128) on 8 trn2 cores.

Strategy: pure data-parallel over batch (4 batches/core, no collectives).
Host folds BN into weights/biases, folds the softmax scale into Wq, and
precomputes the rel-pos-bias table (symmetric). Per core:
  stage1: qkT = Wqk.T @ xT   (head-grouped channel layout, fp32r matmuls,
          DMA striped by n-chunk so PE starts ~3us into the kernel)
          v   = xT.T @ Wv    (token-major layout, fp32r)
  stage2: per (batch, head-pair), heads interleaved for pipelining:
          S = q.kT (K=32 matmuls, packed row groups) + bias injected
          exactly via identity-matmul accumulation on PE; single exp on
          ScalarE whose accum_out is the softmax denominator (logits are
          BN-bounded: no max subtract); P normalized on DVE and
          transposed by bf16 SBUF->SBUF DMA-transpose (PE handles only
          the 16-row sliver); o^T = v.T @ P^T; hardswish as
          h=(t-3)*min(t,6)/6 with t=relu(o+3) kept in f32 (bf16 would
          cancel), min on GpSimd; output projection (bf16, 1/6 folded
          into Wp) emitted per batch after its last head-pair.
"""
import numpy as np
import ml_dtypes

import concourse.bass as bass
import concourse.mybir as mybir
import concourse.tile as tile
from concourse import bacc
from concourse.bass_utils import run_bass_kernel_spmd

F32 = mybir.dt.float32
F32R = mybir.dt.float32r
BF16 = mybir.dt.bfloat16
AF = mybir.ActivationFunctionType
OP = mybir.AluOpType

B, N, DIM = 32, 784, 512
H, KD, VD, RES = 8, 32, 128, 28
NCORES = 8
BL = B // NCORES          # batches per core = 4
NL = BL * N               # 3136 tokens per core
NT = 7                    # n-tiles per batch (6x128 + 16)
MC = 7                    # m-chunks per batch
EPS = 1e-5


def _rows(i):
    return 128 if i < 6 else 16


def build_nc():
    nc = bacc.Bacc(None, target_bir_lowering=False, debug=False)

    xT = nc.dram_tensor("xT", [DIM, NL], F32R, kind="ExternalInput")
    wqk = nc.dram_tensor("wqk", [DIM, 512], F32R, kind="ExternalInput")
    bqk = nc.dram_tensor("bqk", [128, 4], F32, kind="ExternalInput")
    wv = nc.dram_tensor("wv", [DIM, 1024], F32R, kind="ExternalInput")
    bv = nc.dram_tensor("bv", [128, 1024], F32, kind="ExternalInput")
    wp = nc.dram_tensor("wp", [1024, 512], BF16, kind="ExternalInput")
    bp = nc.dram_tensor("bp", [128, 512], F32, kind="ExternalInput")
    eb = nc.dram_tensor("ebias", [H, 896, 784], BF16, kind="ExternalInput")
    ident = nc.dram_tensor("ident", [128, 128], BF16, kind="ExternalInput")
    out = nc.dram_tensor("out", [NL, 512], F32, kind="ExternalOutput")

    with tile.TileContext(nc) as tc:
        with (
            tc.tile_pool(name="persist", bufs=1) as pp,
            tc.tile_pool(name="att", bufs=2) as ap_,
            tc.tile_pool(name="ebp", bufs=1) as ebp,
            tc.tile_pool(name="mmps", bufs=3, space="PSUM") as mm_pool,
        ):
            # persistent SBUF tensors
            qkT = pp.tile([128, 4, NL], BF16)      # q/k channels, head-grouped
            v_sb = pp.tile([128, BL, MC, 1024], BF16)
            wp_sb = pp.tile([128, 8, 512], BF16)
            bp_sb = pp.tile([128, 512], F32)
            id_sb = pp.tile([128, 128], BF16)
            c3 = pp.tile([128, 1], F32)
            nc.vector.memset(c3[:], 3.0)
            nc.sync.dma_start(wp_sb[:], wp[:].rearrange("(hh p) o -> p hh o", p=128))
            nc.sync.dma_start(bp_sb[:], bp[:])
            nc.sync.dma_start(id_sb[:], ident[:])

            # ---------------- stage 1: projections ----------------
            with tc.tile_pool(name="s1", bufs=1) as s1:
                xT_sb = s1.tile([128, 4, NL], F32R)
                wqk_sb = s1.tile([128, 4, 512], F32R)
                wv_sb = s1.tile([128, 4, 1024], F32R)
                bqk_sb = s1.tile([128, 4], F32)
                bv_sb = s1.tile([128, 1024], F32)
                xT_r = xT[:].rearrange("(cc p) n -> p cc n", p=128)
                wqk_r = wqk[:].rearrange("(cc p) o -> p cc o", p=128)
                wv_r = wv[:].rearrange("(cc p) o -> p cc o", p=128)
                for cc in range(4):
                    nc.sync.dma_start(wqk_sb[:, cc], wqk_r[:, cc])
                for ntc in range(NL // 448):
                    for cc in range(4):
                        nc.sync.dma_start(
                            xT_sb[:, cc, ntc * 448:(ntc + 1) * 448],
                            xT_r[:, cc, ntc * 448:(ntc + 1) * 448])
                for cc in range(4):
                    nc.sync.dma_start(wv_sb[:, cc], wv_r[:, cc])
                nc.sync.dma_start(bqk_sb[:], bqk[:])
                nc.sync.dma_start(bv_sb[:], bv[:])

                # qkT[o, n] accumulation over 4 c-chunks; 448-wide n stripes
                for ntc in range(NL // 448):
                    for oc in range(4):
                        ps = mm_pool.tile([128, 512], F32, tag="mm")
                        for cc in range(4):
                            nc.tensor.matmul(
                                ps[:, :448],
                                wqk_sb[:, cc, oc * 128:(oc + 1) * 128],
                                xT_sb[:, cc, ntc * 448:(ntc + 1) * 448],
                                start=(cc == 0), stop=(cc == 3),
                            )
                        nc.scalar.activation(
                            qkT[:, oc, ntc * 448:(ntc + 1) * 448], ps[:, :448],
                            AF.Identity, bias=bqk_sb[:, oc:oc + 1],
                        )

                # v[n, vd] token-major
                for b4 in range(BL):
                    for mc in range(MC):
                        mr = _rows(mc)
                        col0 = b4 * N + mc * 128
                        for vh in range(2):
                            ps = mm_pool.tile([128, 512], F32, tag="mm")
                            for cc in range(4):
                                nc.tensor.matmul(
                                    ps[:mr],
                                    xT_sb[:, cc, col0:col0 + mr],
                                    wv_sb[:, cc, vh * 512:(vh + 1) * 512],
                                    start=(cc == 0), stop=(cc == 3),
                                )
                            nc.vector.tensor_tensor(
                                v_sb[:mr, b4, mc, vh * 512:(vh + 1) * 512],
                                ps[:mr], bv_sb[:mr, vh * 512:(vh + 1) * 512],
                                OP.add,
                            )

            # ---------------- stage 2: attention ----------------
            with (
                tc.tile_pool(name="esp", bufs=3) as esp,
                tc.tile_pool(name="ptp", bufs=1) as ptp,
                tc.tile_pool(name="p2", bufs=1) as pp2,
                tc.tile_pool(name="sps", bufs=1, space="PSUM") as s_pool,
                tc.tile_pool(name="tps", bufs=1, space="PSUM") as t_pool,
            ):
                hT = pp2.tile([128, BL, H, N], BF16)
                for hp in range(4):          # head pairs (2h, 2h+1)
                    ebt = []
                    for i in range(2):
                        h = 2 * hp + i
                        t = ebp.tile([128, MC, 784], BF16, tag=f"eb{i}", name=f"eb{i}")
                        nc.sync.dma_start(
                            t[:], eb[h].rearrange("(mc p) n -> p mc n", p=128))
                        ebt.append(t)
                    for b4 in range(BL):
                        pts = []
                        for i in range(2):
                            ptt = ptp.tile([128, MC, N], BF16, tag=f"pt{i}",
                                           name=f"pt{i}")
                            pts.append(ptt)
                        for nt in range(NT):
                            nr = _rows(nt)
                            c0 = b4 * N + nt * 128
                            nsl = slice(nt * 128, nt * 128 + nr)
                            for i in range(2):
                                h = 2 * hp + i
                                ccq, cck = (0, 1) if h < 4 else (2, 3)
                                pq = 32 * (h % 4)
                                ebh = ebt[i]
                                pt_sb = pts[i]
                                S = s_pool.tile([128, 2, 512], F32, tag=f"s{i}")
                                q = qkT[pq:pq + 32, ccq, c0:c0 + nr]
                                for half in range(2):
                                    m0 = b4 * N + half * 392
                                    nc.tensor.matmul(
                                        S[:nr, half, :392], q,
                                        qkT[pq:pq + 32, cck, m0:m0 + 392],
                                        start=True, stop=False,
                                        tile_position=(pq, 0))
                                    nc.tensor.matmul(
                                        S[:nr, half, :392],
                                        id_sb[:, :nr],
                                        ebh[:, nt, half * 392:half * 392 + 392],
                                        start=False, stop=True)
                                es = esp.tile([128, 784], BF16, tag=f"es{i}")
                                den = ap_.tile([128, 1], F32, tag=f"den{i}")
                                nc.scalar.activation(
                                    es[:nr].rearrange("p (a b) -> p a b", a=2),
                                    S[:nr, :, :392], AF.Exp, accum_out=den[:nr])
                                rd = ap_.tile([128, 1], F32, tag=f"rd{i}")
                                nc.vector.reciprocal(rd[:nr], den[:nr])
                                nc.vector.tensor_scalar_mul(
                                    es[:nr], es[:nr], rd[:nr, 0:1])
                                for mc in range(6):
                                    nc.sync.dma_start_transpose(
                                        pt_sb[:, mc, nsl],
                                        es[:nr, mc * 128:(mc + 1) * 128])
                                ts_ = t_pool.tile([128, 128], F32, tag="pta")
                                nc.tensor.matmul(
                                    ts_[:16, :nr], es[:nr, 768:784],
                                    id_sb[:nr, :nr])
                                nc.vector.tensor_copy(
                                    pt_sb[:16, 6, nsl], ts_[:16, :nr])
                        # AV + hardswish: h = (t-3)*min(t,6)/6, t=relu(o+3)
                        for i in range(2):
                            h = 2 * hp + i
                            pt_sb = pts[i]
                            for hs, hw_ in ((0, 448), (448, 336)):
                                ops = mm_pool.tile([128, 512], F32, tag="mm")
                                for mc in range(MC):
                                    mr = _rows(mc)
                                    nc.tensor.matmul(
                                        ops[:, :hw_],
                                        v_sb[:mr, b4, mc, h * 128:(h + 1) * 128],
                                        pt_sb[:mr, mc, hs:hs + hw_],
                                        start=(mc == 0), stop=(mc == 6))
                                tt = ap_.tile([128, 512], F32, tag="hsw")
                                if h % 2 == 0:
                                    nc.vector.tensor_scalar(
                                        tt[:, :hw_], ops[:, :hw_], 3.0, 0.0,
                                        OP.add, OP.max)
                                else:
                                    nc.scalar.activation(
                                        tt[:, :hw_], ops[:, :hw_], AF.Relu,
                                        bias=c3[:, 0:1])
                                mm_ = ap_.tile([128, 512], F32, tag="mmin")
                                nc.gpsimd.tensor_scalar_min(
                                    mm_[:, :hw_], tt[:, :hw_], 6.0)
                                nc.vector.scalar_tensor_tensor(
                                    hT[:, b4, h, hs:hs + hw_],
                                    tt[:, :hw_], -3.0, mm_[:, :hw_],
                                    OP.add, OP.mult)
                        if hp == 3:
                            # all 8 heads of b4 done: output projection
                            for nt in range(NT):
                                nr = _rows(nt)
                                c0 = b4 * N + nt * 128
                                ps = mm_pool.tile([128, 512], F32, tag="mm")
                                for hh in range(8):
                                    nc.tensor.matmul(
                                        ps[:nr],
                                        hT[:, b4, hh, nt * 128:nt * 128 + nr],
                                        wp_sb[:, hh, :],
                                        start=(hh == 0), stop=(hh == 7))
                                ob = ap_.tile([128, 512], F32, tag="ob")
                                nc.vector.tensor_tensor(ob[:nr], ps[:nr],
                                                        bp_sb[:nr], OP.add)
                                nc.sync.dma_start(out[c0:c0 + nr, :], ob[:nr])

    nc.compile()
    return nc


_NC = None


def _prep_weights(qkv_w, qkv_g, qkv_b, qkv_m, qkv_v, ab, proj_w, proj_g,
                  proj_b, proj_m, proj_v, idxs):
    s = qkv_g / np.sqrt(qkv_v + EPS)
    W = qkv_w * s[:, None]                       # [1536, 512]
    bias = qkv_b - qkv_m * s                     # [1536]
    scale = KD ** -0.5
    # head-grouped reorder: chunk0=q0..3, chunk1=k0..3, chunk2=q4..7, chunk3=k4..7
    qk_rows, v_rows = [], []
    for h in range(H):
        base = h * (2 * KD + VD)
        qk_rows.append((np.arange(base, base + KD), True))
        qk_rows.append((np.arange(base + KD, base + 2 * KD), False))
        v_rows.append(np.arange(base + 2 * KD, base + 2 * KD + VD))
    order = []
    for grp in range(4):
        half = grp // 2
        is_q = (grp % 2 == 0)
        for hh in range(4 * half, 4 * half + 4):
            order.append((qk_rows[2 * hh][0] if is_q else qk_rows[2 * hh + 1][0], is_q))
    wqk = np.empty((512, 512), np.float32)
    bqk = np.empty(512, np.float32)
    o = 0
    for rows, is_q in order:
        f = scale if is_q else 1.0
        wqk[:, o:o + KD] = (W[rows] * f).T
        bqk[o:o + KD] = bias[rows] * f
        o += KD
    vr = np.concatenate(v_rows)
    wv = W[vr].T.copy()                          # [512, 1024]
    bv = bias[vr]

    sp = proj_g / np.sqrt(proj_v + EPS)
    # reference: out = h @ proj_w.T * sp + (proj_b - proj_m*sp); fold 1/6 of hswish
    wp = (proj_w * sp[:, None]).T.astype(np.float32) / 6.0   # [1024, 512]
    bp = proj_b - proj_m * sp

    btab = ab[:, idxs].astype(np.float32)                    # [H, 784, 784]
    eb_pad = np.zeros((H, 896, 784), np.float32)
    eb_pad[:, :784] = btab

    return dict(
        wqk=wqk, bqk=bqk.reshape(4, 128).T.copy(),
        wv=wv, bv=np.broadcast_to(bv, (128, 1024)).copy(),
        wp=wp.astype(ml_dtypes.bfloat16),
        bp=np.broadcast_to(bp, (128, 512)).astype(np.float32).copy(),
        ebias=eb_pad.astype(ml_dtypes.bfloat16),
        ident=np.eye(128, dtype=ml_dtypes.bfloat16),
    )


def kernel(x, qkv_w, qkv_g, qkv_b, qkv_m, qkv_v, ab,
           proj_w, proj_g, proj_b, proj_m, proj_v, idxs, _trace=False):
    global _NC
    x = np.asarray(x, np.float32)
    shared = _prep_weights(
        np.asarray(qkv_w, np.float32), np.asarray(qkv_g, np.float32),
        np.asarray(qkv_b, np.float32), np.asarray(qkv_m, np.float32),
        np.asarray(qkv_v, np.float32), np.asarray(ab, np.float32),
        np.asarray(proj_w, np.float32), np.asarray(proj_g, np.float32),
        np.asarray(proj_b, np.float32), np.asarray(proj_m, np.float32),
        np.asarray(proj_v, np.float32), np.asarray(idxs))

    if _NC is None:
        _NC = build_nc()
    nc = _NC

    in_maps = []
    for c in range(NCORES):
        xs = x[c * BL:(c + 1) * BL]                      # [4, 784, 512]
        xT = xs.transpose(2, 0, 1).reshape(DIM, NL).copy()
        m = dict(shared)
        m["xT"] = xT
        in_maps.append(m)

    res = run_bass_kernel_spmd(nc, in_maps, core_ids=list(range(NCORES)),
                               trace=_trace)
    outs = [res.results[c]["out"].reshape(BL, N, DIM) for c in range(NCORES)]
    full = np.concatenate(outs, axis=0)
    if _trace:
        return full, res.exec_time_ns
    return full
